# revision 26
# baseline (speedup 1.0000x reference)
"""Trainium2 Bass kernel for a batched LSTM + per-step 2-class sigmoid head.

Model (reference): x = concat(points, times) [B,T,24]; LSTM(HS=128) over T;
out = sigmoid(h_seq @ Wc + bc) [B,T,2].  B=512, T=1024.

Strategy: pure data parallel over batch (64 rows per core, 8 cores).
Per core, gate-major on-chip layout:
  - state h,c: [HS=128 partitions, 64 batch]
  - gates for a window of TAU=8 steps live in PSUM as [128, 4*TAU*64] f32,
    one PSUM bank per gate (order i,f,o,g). Window x@W matmuls pre-fill the
    banks (start=True); per-step U@h matmuls accumulate on top (start=False).
  - one Sigmoid ACT covers i,f,o via a strided AP over 3 banks; Tanh for g;
    DVE does c = f*c + i*g; Tanh(c); h = o*tanh(c) written directly as bf16.
  - per-step classifier matmul h@Wc into a rolling PSUM bank; every 256 steps
    one Sigmoid (+bc bias) pass and one contiguous DMA to the output.
Host-side prep: fold bias into W via an all-ones input row, permute gate
columns to (i,f,o,g), transpose x to [25, T*64], cast matmul operands to bf16.
"""

import os
import numpy as np
import ml_dtypes
from contextlib import ExitStack

HS = 128
INP = 23
NCORES = 8

# variant knobs (env-overridable for A/B testing)
CHUNKS = int(os.environ.get("LSTM_CHUNKS", "2"))
FUSE_G = os.environ.get("LSTM_FUSE_G", "1") == "1"
T1_GPSIMD = os.environ.get("LSTM_T1_GPSIMD", "1") == "1"
H_GPSIMD = os.environ.get("LSTM_H_GPSIMD", "1") == "1"
TAU = int(os.environ.get("LSTM_TAU", "4"))
XW_SPLIT = os.environ.get("LSTM_XW_SPLIT", "1") == "1"
MM_FIRST = os.environ.get("LSTM_MM_FIRST", "0") == "1"
H_DEFER = os.environ.get("LSTM_H_DEFER", "0") == "1"
TANH_ALL = os.environ.get("LSTM_TANH_ALL", "0") == "1"

_BUILD_CACHE = {}


def build_lstm(T=1024, BL=64, chunks=1, tau=8, fuse_g=False, t1_gpsimd=False,
               h_gpsimd=False, xw_split=False, mm_first=False, h_defer=False,
               tanh_all=False):
    """Build the Bass module for one core (SPMD: all cores identical).

    fuse_g: host pre-scales the g columns of W/U by 2 so one Sigmoid ACT
        covers all 4 gates (sigmoid(2a) = (tanh(a)+1)/2); the c-update uses a
        fused scalar_tensor_tensor to apply g = 2*s - 1.
    t1_gpsimd: compute f*c on GPSIMD to offload the DVE.
    """
    import concourse.bacc as bacc
    import concourse.tile as tile
    from concourse import mybir

    f32 = mybir.dt.float32
    bf16 = mybir.dt.bfloat16
    Sig = mybir.ActivationFunctionType.Sigmoid
    Tanh = mybir.ActivationFunctionType.Tanh

    assert T % tau == 0 and BL % chunks == 0
    CW = BL // chunks
    TB = tau * BL            # free-dim size of one gate's window region
    NW = T // tau
    CLS_STEPS = min(256, T)  # steps per classifier PSUM bank (2 cols per step)
    assert CLS_STEPS % tau == 0 and T % CLS_STEPS == 0

    nc = bacc.Bacc("TRN2", target_bir_lowering=False, debug=False)

    xt_d = nc.dram_tensor("xt", [INP + 2, T * BL], bf16, kind="ExternalInput")
    u_d = nc.dram_tensor("u", [HS, 4 * HS], bf16, kind="ExternalInput")
    w_d = nc.dram_tensor("w", [INP + 2, 4 * HS], bf16, kind="ExternalInput")
    wc_d = nc.dram_tensor("wc", [HS, 2], bf16, kind="ExternalInput")
    bc_d = nc.dram_tensor("bc", [BL, 2], f32, kind="ExternalInput")
    out_d = nc.dram_tensor("out", [BL, T * 2], f32, kind="ExternalOutput")

    with ExitStack() as ctx:
        tc = ctx.enter_context(tile.TileContext(nc))
        consts = ctx.enter_context(tc.tile_pool(name="consts", bufs=1))
        state = ctx.enter_context(tc.tile_pool(name="state", bufs=3))
        gwork = ctx.enter_context(tc.tile_pool(name="gwork", bufs=3))
        outp = ctx.enter_context(tc.tile_pool(name="outp", bufs=2))
        gates_bufs = 2 if tau <= 4 else 1
        gates_ps = ctx.enter_context(
            tc.tile_pool(name="gates_ps", bufs=gates_bufs, space="PSUM"))
        cls_ps = ctx.enter_context(tc.tile_pool(name="cls_ps", bufs=2, space="PSUM"))

        # ---- load constants / inputs into SBUF ----
        xt_sb = consts.tile([INP + 2, T * BL], bf16)
        n_dma = 4
        sl = T * BL // n_dma
        for i in range(n_dma):
            nc.sync.dma_start(
                out=xt_sb[:, i * sl:(i + 1) * sl], in_=xt_d.ap()[:, i * sl:(i + 1) * sl]
            )
        u_sb = consts.tile([HS, 4 * HS], bf16)
        nc.sync.dma_start(out=u_sb, in_=u_d.ap())
        w_sb = consts.tile([INP + 2, 4 * HS], bf16)
        nc.sync.dma_start(out=w_sb, in_=w_d.ap())
        wc_sb = consts.tile([HS, 2], bf16)
        nc.sync.dma_start(out=wc_sb, in_=wc_d.ap())
        bc_sb = consts.tile([BL, 2], f32)
        nc.sync.dma_start(out=bc_sb, in_=bc_d.ap())

        # ---- initial state ----
        c_prev = []
        h_prev = []
        for ch in range(chunks):
            if tanh_all:
                c0t = state.tile([HS, CW, 3], f32, tag=f"c{ch}")
                nc.vector.memset(c0t, 0.0)
                c0 = c0t[:, :, 2]
            else:
                c0 = state.tile([HS, CW], f32, tag=f"c{ch}")
                nc.vector.memset(c0, 0.0)
            h0 = state.tile([HS, CW], bf16, tag=f"h{ch}")
            nc.vector.memset(h0, 0.0)
            c_prev.append(c0)
            h_prev.append(h0)

        from concourse.tile_rust import add_dep_helper

        xw_bank_first = {}   # window -> bank-clearing matmul (for race checker)

        def phase_a_gate(w, gp, gc):
            # pre-fill one gate region of window w's PSUM with x@W (+bias via
            # ones row).  start=True clears has_written for the whole 2KB bank,
            # so only the first gate region per bank uses it; later regions in
            # the same bank use start=False (bits clear -> overwrite) and must
            # be ordered after the bank-clearing matmul (PE is in-order and we
            # emit in gate order, so only the race checker needs the hint).
            x_sl = xt_sb[:, w * TB:(w + 1) * TB]
            is_first = (gc * TB * 4) % 2048 == 0
            mm = nc.tensor.matmul(
                out=gp[:, gc * TB:(gc + 1) * TB],
                lhsT=w_sb[:, gc * HS:(gc + 1) * HS],
                rhs=x_sl,
                start=is_first,
                stop=False,
                skip_group_check=True,
            )
            if is_first:
                xw_bank_first[w] = mm
            else:
                add_dep_helper(mm.ins, xw_bank_first[w].ins, sync=False,
                               reason="bank-clear order")

        def phase_a(w, gp):
            for gc in range(4):
                phase_a_gate(w, gp, gc)

        mul_engine = nc.gpsimd if t1_gpsimd else nc.vector

        if tanh_all:
            # static scan multipliers [0, 1, 0.5] and per-chunk [A, B, 0]
            # operand tiles for the 3-phase (A+B)/2 tensor_tensor_scan
            scan_d0 = consts.tile([HS, CW, 3], f32, name="scan_d0")
            nc.vector.memset(scan_d0, 0.0)
            nc.vector.memset(scan_d0[:, :, 1], 1.0)
            nc.vector.memset(scan_d0[:, :, 2], 0.5)
            scan_d1 = []
            for ch in range(chunks):
                d1 = consts.tile([HS, CW, 3], f32, name=f"scan_d1_{ch}")
                nc.vector.memset(d1, 0.0)
                scan_d1.append(d1)

        def step_activations(gp_r, s, bsl, ch, c_prev_t):
            if tanh_all:
                # ONE Tanh ACT covers all 4 gates: host pre-scales i,f,o
                # columns by 0.5 so tau_x = tanh(a_x/2) = 2*sigmoid(a_x)-1;
                # g is exact tanh.  A = (tau_f+1)*c = 2f*c and
                # B = (tau_i+1)*tau_g = 2i*g via two STTs written into slots
                # 0/1 of the scan operand; the 3-phase scan computes
                # c = (A+B)/2 per batch element: state = A; state += B;
                # state *= 0.5.  All on DVE, no GPSIMD on the chain.
                # h is carried as H = 2h = (tau_o+1)*tanh(c) with U, Wc
                # pre-halved on the host.
                tg = gwork.tile([HS, 4, CW], f32, tag=f"sg{ch}")
                nc.scalar.activation(out=tg, in_=gp_r[:, 0:4, s, bsl], func=Tanh)
                d1 = scan_d1[ch]
                nc.vector.scalar_tensor_tensor(
                    d1[:, :, 1], tg[:, 0, :], 1.0, tg[:, 3, :],
                    mybir.AluOpType.add, mybir.AluOpType.mult)
                nc.vector.scalar_tensor_tensor(
                    d1[:, :, 0], tg[:, 1, :], 1.0, c_prev_t,
                    mybir.AluOpType.add, mybir.AluOpType.mult)
                c_tile = state.tile([HS, CW, 3], f32, tag=f"c{ch}")
                flat = "p b s -> p (b s)"
                nc.vector.tensor_tensor_scan(
                    out=c_tile.rearrange(flat), data0=scan_d0.rearrange(flat),
                    data1=d1.rearrange(flat), initial=0.0,
                    op0=mybir.AluOpType.mult, op1=mybir.AluOpType.add)
                c_new = c_tile[:, :, 2]
                sg = tg
            elif fuse_g:
                # one Sigmoid over all 4 gates; g columns pre-scaled by 2 so
                # slot 3 holds s with tanh(a_g) = 2*s - 1
                sg = gwork.tile([HS, 4, CW], f32, tag=f"sg{ch}")
                nc.scalar.activation(out=sg, in_=gp_r[:, 0:4, s, bsl], func=Sig)
                t1 = gwork.tile([HS, CW], f32, tag=f"t1{ch}")
                mul_engine.tensor_mul(t1, sg[:, 1, :], c_prev_t)
                t2 = gwork.tile([HS, CW], f32, tag=f"t2{ch}")
                nc.vector.tensor_mul(t2, sg[:, 0, :], sg[:, 3, :])   # i * s
                # t3 = 2*(i*s) - i  ( = i * (2s-1) = i * tanh(a_g) )
                t3 = gwork.tile([HS, CW], f32, tag=f"t3{ch}")
                nc.vector.scalar_tensor_tensor(
                    t3, t2, 2.0, sg[:, 0, :],
                    mybir.AluOpType.mult, mybir.AluOpType.subtract,
                )
                c_new = state.tile([HS, CW], f32, tag=f"c{ch}")
                nc.vector.tensor_add(c_new, t1, t3)
            else:
                # sigmoid over i,f,o (strided 3-bank AP), tanh for g
                sg = gwork.tile([HS, 3, CW], f32, tag=f"sg{ch}")
                nc.scalar.activation(out=sg, in_=gp_r[:, 0:3, s, bsl], func=Sig)
                gt = gwork.tile([HS, CW], f32, tag=f"g{ch}")
                nc.scalar.activation(out=gt, in_=gp_r[:, 3, s, bsl], func=Tanh)
                t1 = gwork.tile([HS, CW], f32, tag=f"t1{ch}")
                mul_engine.tensor_mul(t1, sg[:, 1, :], c_prev_t)
                t2 = gwork.tile([HS, CW], f32, tag=f"t2{ch}")
                nc.vector.tensor_mul(t2, sg[:, 0, :], gt)
                c_new = state.tile([HS, CW], f32, tag=f"c{ch}")
                nc.vector.tensor_add(c_new, t1, t2)
            m = gwork.tile([HS, CW], f32, tag=f"m{ch}")
            nc.scalar.activation(out=m, in_=c_new, func=Tanh)
            if h_defer:
                # caller emits h = o*tanh(c) after all chunks' c-blocks so no
                # engine queue has one chunk's h ahead of the other's c ops
                return c_new, (sg[:, 2, :], m)
            # h = o * tanh(c), produced directly as bf16 for the matmuls
            h_new = state.tile([HS, CW], bf16, tag=f"h{ch}")
            (nc.gpsimd if h_gpsimd else nc.vector).tensor_mul(h_new, sg[:, 2, :], m)
            return c_new, h_new

        cp = [None] * chunks           # per-chunk classifier PSUM tile
        h_cls = [None] * chunks        # h tile of step t-1 awaiting its cls MM

        def emit_cls(t, ch):
            # classifier matmul for step t (deferred one step so it doesn't
            # sit on the critical chain ahead of the next step's U matmuls)
            r = t % CLS_STEPS
            if r == 0:
                cp[ch] = cls_ps.tile([CW, 2 * CLS_STEPS], f32, tag=f"cp{ch}",
                                     name=f"cp{ch}")
            nc.tensor.matmul(
                out=cp[ch][:, 2 * r:2 * r + 2],
                lhsT=h_cls[ch],
                rhs=wc_sb,
                start=(r == 0),
                stop=(r == CLS_STEPS - 1),
                skip_group_check=True,
            )
            if r == CLS_STEPS - 1:
                # end of a classifier block: sigmoid(+bc) and DMA out
                blk = t // CLS_STEPS
                ob = outp.tile([CW, 2 * CLS_STEPS], f32, tag=f"ob{ch}")
                cp_r = cp[ch].rearrange("p (s c) -> p s c", c=2)
                ob_r = ob.rearrange("p (s c) -> p s c", c=2)
                for cls in range(2):
                    nc.scalar.activation(
                        out=ob_r[:, :, cls],
                        in_=cp_r[:, :, cls],
                        func=Sig,
                        bias=bc_sb[0:CW, cls:cls + 1],
                    )
                nc.sync.dma_start(
                    out=out_d.ap()[ch * CW:(ch + 1) * CW,
                                   blk * 2 * CLS_STEPS:(blk + 1) * 2 * CLS_STEPS],
                    in_=ob,
                )

        gp_cur = gates_ps.tile([HS, 4 * TB], f32, tag="gates")
        phase_a(0, gp_cur)
        for w in range(NW):
            gp_r = gp_cur.rearrange("p (g s b) -> p g s b", g=4, s=tau)
            gp_next = None
            if w + 1 < NW:
                gp_next = gates_ps.tile([HS, 4 * TB], f32, tag="gates")
            for s in range(tau):
                t = w * tau + s

                def emit_umm(ch):
                    for gc in range(4):
                        nc.tensor.matmul(
                            out=gp_cur[:, gc * TB + s * BL + ch * CW:
                                       gc * TB + s * BL + (ch + 1) * CW],
                            lhsT=u_sb[:, gc * HS:(gc + 1) * HS],
                            rhs=h_prev[ch],
                            start=False,
                            stop=(s == tau - 1 and ch == chunks - 1),
                            skip_group_check=True,
                        )

                if mm_first:
                    # chain-critical U matmuls for ALL chunks first in PE queue
                    for ch in range(chunks):
                        emit_umm(ch)
                pend = [None] * chunks
                for ch in range(chunks):
                    bsl = slice(ch * CW, (ch + 1) * CW)
                    if not mm_first:
                        emit_umm(ch)
                    # previous step's classifier matmul fills the PE gap here
                    if h_cls[ch] is not None:
                        emit_cls(t - 1, ch)
                    c_new, h_new = step_activations(gp_r, s, bsl, ch, c_prev[ch])
                    c_prev[ch] = c_new
                    if h_defer:
                        pend[ch] = h_new   # (o_ap, m_tile)
                    else:
                        h_prev[ch] = h_new
                        h_cls[ch] = h_new
                if h_defer:
                    for ch in range(chunks):
                        o_ap, m = pend[ch]
                        h_new = state.tile([HS, CW], bf16, tag=f"h{ch}")
                        if tanh_all:
                            # H = 2h = (tau_o + 1) * tanh(c)
                            nc.vector.scalar_tensor_tensor(
                                h_new, o_ap, 1.0, m,
                                mybir.AluOpType.add, mybir.AluOpType.mult)
                        else:
                            (nc.gpsimd if h_gpsimd else nc.vector).tensor_mul(
                                h_new, o_ap, m)
                        h_prev[ch] = h_new
                        h_cls[ch] = h_new
                # double-buffered windows: emit next window's x@W mid-window,
                # spread over steps s>=1 (at s=0 the WAR against the previous
                # window's in-flight sigmoid reads would stall the PE queue)
                if gp_next is not None and gates_bufs > 1:
                    if xw_split and tau == 4:
                        gates_at = {1: [0, 1], 2: [2], 3: [3]}.get(s, [])
                        for gc in gates_at:
                            phase_a_gate(w + 1, gp_next, gc)
                    elif s == 1:
                        phase_a(w + 1, gp_next)
            if gp_next is not None and gates_bufs == 1:
                phase_a(w + 1, gp_next)
            if gp_next is not None:
                gp_cur = gp_next
        for ch in range(chunks):
            emit_cls(T - 1, ch)
    nc.compile()
    return nc


def _prep_inputs(points, times, W, U, bias, Wc, bc, T, BL, ncores, fuse_g=False,
                 tanh_all=False):
    """Host-side prep: permute gates to (i,f,o,g), fold bias via ones row,
    transpose x to [25, T*BL] per core, cast matmul operands to bf16."""
    bf = ml_dtypes.bfloat16
    perm = np.concatenate([np.r_[0:HS], np.r_[HS:2 * HS], np.r_[3 * HS:4 * HS],
                           np.r_[2 * HS:3 * HS]])
    Wp = np.concatenate([W, bias[None, :]], axis=0)[:, perm]      # [25, 512]
    Up = U[:, perm]                                                # [128, 512]
    if tanh_all:
        # tau_x = tanh(a_x/2) = 2*sigmoid(a_x)-1 for i,f,o (g exact tanh);
        # the recurrent input is H = 2h, so U absorbs an extra 0.5, and the
        # classifier weight absorbs 0.5 for h = H/2.
        Wp = Wp.copy()
        Up = Up.copy()
        Wp[:, :3 * HS] *= 0.5
        Up[:, :3 * HS] *= 0.5
        Up *= 0.5
        Wc = Wc * 0.5
    elif fuse_g:
        Wp = Wp.copy()
        Up = Up.copy()
        Wp[:, 3 * HS:] *= 2.0    # g columns now produce 2*a_g
        Up[:, 3 * HS:] *= 2.0
    x = np.concatenate([points, times[..., None]], axis=-1)        # [B, T, 24]

    u_bf = np.ascontiguousarray(Up).astype(bf)
    w_bf = np.ascontiguousarray(Wp).astype(bf)                     # [25, 512]
    wc_bf = np.ascontiguousarray(Wc).astype(bf)
    bc_f = np.ascontiguousarray(np.broadcast_to(bc[None, :], (BL, 2))).astype(np.float32)

    in_maps = []
    for k in range(ncores):
        xs = x[k * BL:(k + 1) * BL, :T]                            # [BL, T, 24]
        xt = np.empty((INP + 2, T * BL), dtype=bf)
        xt[:INP + 1] = xs.transpose(2, 1, 0).reshape(INP + 1, T * BL).astype(bf)
        xt[INP + 1] = np.ones((), dtype=bf)                        # bias ones row
        in_maps.append({"xt": xt, "u": u_bf, "w": w_bf, "wc": wc_bf, "bc": bc_f})
    return in_maps


def kernel(points, times, W, U, bias, Wc, bc, _run_kwargs=None):
    from concourse.bass_utils import run_bass_kernel_spmd

    B, T = times.shape
    BL = B // NCORES
    key = (T, BL, CHUNKS, TAU, FUSE_G, T1_GPSIMD, H_GPSIMD, XW_SPLIT, MM_FIRST,
           H_DEFER, TANH_ALL)
    if key not in _BUILD_CACHE:
        _BUILD_CACHE[key] = build_lstm(T=T, BL=BL, chunks=CHUNKS, tau=TAU,
                                       fuse_g=FUSE_G, t1_gpsimd=T1_GPSIMD,
                                       h_gpsimd=H_GPSIMD, xw_split=XW_SPLIT,
                                       mm_first=MM_FIRST,
                                       h_defer=H_DEFER or TANH_ALL,
                                       tanh_all=TANH_ALL)
    nc = _BUILD_CACHE[key]

    in_maps = _prep_inputs(points, times, W, U, bias, Wc, bc, T, BL, NCORES,
                           fuse_g=FUSE_G, tanh_all=TANH_ALL)
    kw = _run_kwargs or {}
    res = run_bass_kernel_spmd(nc, in_maps, core_ids=list(range(NCORES)), **kw)
    out = np.concatenate(
        [r["out"].reshape(BL, T, 2) for r in res.results], axis=0
    ).astype(np.float32)
    if _run_kwargs is not None:
        return out, res
    return out

